# revision 12
# baseline (speedup 1.0000x reference)
"""Trainium2 Bass kernel for nn_EquiConv (e3nn-style FullyConnectedTensorProduct
+ gate + radial-MLP elementwise conv), data-parallel over edges on 8 cores.

v3 design:
  - ss/vv paths: per-edge kron built on DVE (2x pair APs) -> PE transpose
    -> single ACT PSUM->SBUF copy per 4-chunk stage -> accumulating matmuls.
  - sv/vs paths restructured as two-stage contraction: shared-weight matmul
    B_sv[e,(w,v)] = sum_u x1s[e,u] Wsv[u,(w,v)] (K=64, N=1024, edge-major out),
    then per-edge product (broadcast over middle dim, 2x) + packed reduce over
    the 64-wide (v|u) axis on DVE. Replaces 96 chunk matmuls + 192 transposes
    per supertile with 4 matmuls.
  - Silu computed as x*sigmoid(x) so only the sigmoid act-table set is ever
    loaded (one LoadActFuncSet total).
  - All per-edge inputs are host-prearranged bf16 in the exact consumed
    layouts; vec output stays edge-major through gating and is written
    edge-major (host transposes for free).
"""

import sys

sys.path.insert(0, "/opt/trn_rl_repo")

import numpy as np
import ml_dtypes

import concourse.bass as bass
import concourse.bacc as bacc
import concourse.mybir as mybir
import concourse.tile as tile
from concourse.bass_utils import run_bass_kernel_spmd

BF16 = ml_dtypes.bfloat16

E = 20000
S = 64
V = 32
FC_IN = 128
HID = 64
INV_SQRT3 = 0.5773502691896258

NCORES = 8
EC = E // NCORES  # 2500 edges per core
ET = 128  # edges per tile
NT = (EC + ET - 1) // ET  # 20 tiles
EPAD = NT * ET  # 2560

A_SC = float(1.0 / np.sqrt(np.float32(S * S + V * V)))
A_VEC = float(1.0 / np.sqrt(np.float32(2 * S * V)))

f32 = mybir.dt.float32
bf16 = mybir.dt.bfloat16

N_SS = (S * S) // 128  # 32 ss chunks -> 96-wide out (sc|g)
N_VV = (V * V) // 128  # 8 vv chunks  -> same out


def _prep_weights(w_ss_s, w_vv_s, w_ss_g, w_vv_g, w_sv_v, w_vs_v,
                  fc_w1, fc_b1, fc_w2, fc_b2, fc_w3, fc_b3):
    """Host-side rearrangement of the shared weights."""
    wss = np.concatenate([w_ss_s, w_ss_g], axis=2) * A_SC  # [64,64,96]
    wvv = np.concatenate([w_vv_s, w_vv_g], axis=2) * (A_SC * INV_SQRT3)  # [32,32,96]
    w_ssvv = np.concatenate(
        [wss.reshape(S * S, S + V), wvv.reshape(V * V, S + V)], axis=0
    )  # [5120, 96];  k = u*64+v (ss) ++ 4096 + u*32+v (vv)
    w_ssvv = (
        w_ssvv.reshape(N_SS + N_VV, 128, S + V).transpose(1, 0, 2)
        .reshape(128, (N_SS + N_VV) * (S + V))
    )

    # Wsv[u, w*32+v] = w_sv_v[u,v,w]*A_VEC  (w-major cols so B is [e,(w,v)])
    wsv = (w_sv_v * A_VEC).transpose(0, 2, 1).reshape(S, V * V)
    # Wvs[v, w*32+u] = w_vs_v[u,v,w]*A_VEC
    wvs = (w_vs_v * A_VEC).transpose(1, 2, 0).reshape(S, V * V)

    return {
        "w_ssvv": w_ssvv.astype(BF16),
        "w_sv": wsv.astype(BF16),  # [64, 1024]
        "w_vs": wvs.astype(BF16),  # [64, 1024]
        "w_fc1": fc_w1.astype(BF16),  # [128, 64]
        "w_fc2": fc_w2.astype(BF16),  # [64, 64]
        "w_fc3": fc_w3.astype(BF16),  # [64, 96]
        "b_fc1": fc_b1.reshape(HID, 1).astype(np.float32),
        "b_fc2": fc_b2.reshape(HID, 1).astype(np.float32),
        "b_fc3": fc_b3.reshape(S + V, 1).astype(np.float32),
        "ident": np.eye(128, dtype=BF16),
    }


def _build_program():
    nc = bacc.Bacc("TRN2", target_bir_lowering=False, debug=False)

    # per-edge inputs, host-prearranged bf16
    # xcat cols: x1s2 [0:128] | x2s [128:192] | x1vg2 [192:384] | xvgcat [384:576]
    # xvgcat[e, i, 0:32] = x2v[e,:,i], [e, i, 32:64] = x1v[e,:,i]
    d_xcat = nc.dram_tensor("xcat", [EPAD, 576], bf16, kind="ExternalInput").ap()
    d_x1sT = nc.dram_tensor("x1sT", [S, EPAD], bf16, kind="ExternalInput").ap()
    d_x2sT = nc.dram_tensor("x2sT", [S, EPAD], bf16, kind="ExternalInput").ap()
    d_fwT = nc.dram_tensor("fwT", [FC_IN, EPAD], bf16, kind="ExternalInput").ap()
    # shared weights
    d_wssvv = nc.dram_tensor("w_ssvv", [128, (N_SS + N_VV) * (S + V)], bf16, kind="ExternalInput").ap()
    d_wsv = nc.dram_tensor("w_sv", [S, V * V], bf16, kind="ExternalInput").ap()
    d_wvs = nc.dram_tensor("w_vs", [S, V * V], bf16, kind="ExternalInput").ap()
    d_wfc1 = nc.dram_tensor("w_fc1", [FC_IN, HID], bf16, kind="ExternalInput").ap()
    d_wfc2 = nc.dram_tensor("w_fc2", [HID, HID], bf16, kind="ExternalInput").ap()
    d_wfc3 = nc.dram_tensor("w_fc3", [HID, S + V], bf16, kind="ExternalInput").ap()
    d_bfc1 = nc.dram_tensor("b_fc1", [HID, 1], f32, kind="ExternalInput").ap()
    d_bfc2 = nc.dram_tensor("b_fc2", [HID, 1], f32, kind="ExternalInput").ap()
    d_bfc3 = nc.dram_tensor("b_fc3", [S + V, 1], f32, kind="ExternalInput").ap()
    d_ident = nc.dram_tensor("ident", [128, 128], bf16, kind="ExternalInput").ap()

    d_osc = nc.dram_tensor("out_sc", [S, EPAD], f32, kind="ExternalOutput").ap()
    d_ovec = nc.dram_tensor("out_vec", [EPAD, 3 * V], f32, kind="ExternalOutput").ap()

    Sigm = mybir.ActivationFunctionType.Sigmoid
    Copy = mybir.ActivationFunctionType.Copy
    Ident = mybir.ActivationFunctionType.Identity
    mul_op = mybir.AluOpType.mult
    add_op = mybir.AluOpType.add

    NSUB = 2
    EW = NSUB * ET              # 256 edges per supertile
    NSUP = EPAD // EW           # 10
    NCH = N_SS + N_VV           # 40 chunks (ss+vv only)
    CPG = 8                     # chunks per stage group

    # chunk metadata: (kron tag, k-offset, w col off)
    chunk_meta = []
    for c in range(N_SS):
        chunk_meta.append(("kss", c * 128, c * (S + V)))
    for c in range(N_VV):
        chunk_meta.append(("kvv", c * 128, (N_SS + c) * (S + V)))

    def silu(postp, out_tile, in_ap, bias, tag, mul_eng, uniq):
        """out = silu(in+bias) = (in+bias)*sigmoid(in+bias), Sigmoid-set only."""
        shp = [out_tile[:].shape[0], out_tile[:].shape[1]]
        xm = postp.tile(shp, bf16, tag=tag + "_xm", name=uniq + "_xm")
        sg = postp.tile(shp, bf16, tag=tag + "_sg", name=uniq + "_sg")
        if bias is None:
            nc.scalar.activation(xm[:], in_ap, Copy)
            nc.scalar.activation(sg[:], in_ap, Sigm)
        else:
            nc.scalar.activation(xm[:], in_ap, Ident, bias=bias)
            nc.scalar.activation(sg[:], in_ap, Sigm, bias=bias)
        mul_eng.tensor_tensor(out_tile[:], xm[:], sg[:], mul_op)

    with tile.TileContext(nc) as tc:
        with (
            tc.tile_pool(name="consts", bufs=1) as consts,
            tc.tile_pool(name="io", bufs=2) as io,
            tc.tile_pool(name="kron", bufs=2) as kronp,
            tc.tile_pool(name="pp", bufs=1) as ppp,
            tc.tile_pool(name="stage", bufs=3) as stagep,
            tc.tile_pool(name="post", bufs=2) as postp,
            tc.tile_pool(name="pst", bufs=1, space=bass.MemorySpace.PSUM) as pst,
            tc.tile_pool(name="pacc", bufs=1, space=bass.MemorySpace.PSUM) as pacc,
            tc.tile_pool(name="pb", bufs=1, space=bass.MemorySpace.PSUM) as pb,
            tc.tile_pool(name="pmlp", bufs=1, space=bass.MemorySpace.PSUM) as pmlp,
        ):
            # ---- constants / resident inputs ----
            wssvv = consts.tile([128, NCH * (S + V)], bf16, name="wssvv")
            nc.scalar.dma_start(wssvv[:], d_wssvv)
            wsv = consts.tile([S, V * V], bf16, name="wsv")
            nc.scalar.dma_start(wsv[:], d_wsv)
            wvs = consts.tile([S, V * V], bf16, name="wvs")
            nc.scalar.dma_start(wvs[:], d_wvs)
            wfc1 = consts.tile([FC_IN, HID], bf16)
            nc.scalar.dma_start(wfc1[:], d_wfc1)
            wfc2 = consts.tile([HID, HID], bf16)
            nc.scalar.dma_start(wfc2[:], d_wfc2)
            wfc3 = consts.tile([HID, S + V], bf16)
            nc.scalar.dma_start(wfc3[:], d_wfc3)
            bfc1 = consts.tile([HID, 1], f32)
            nc.scalar.dma_start(bfc1[:], d_bfc1)
            bfc2 = consts.tile([HID, 1], f32)
            nc.scalar.dma_start(bfc2[:], d_bfc2)
            bfc3 = consts.tile([S + V, 1], f32)
            nc.scalar.dma_start(bfc3[:], d_bfc3)
            ident = consts.tile([128, 128], bf16)
            nc.scalar.dma_start(ident[:], d_ident)
            x1sT = consts.tile([S, EPAD], bf16, name="x1sT")
            nc.scalar.dma_start(x1sT[:], d_x1sT)
            x2sT = consts.tile([S, EPAD], bf16, name="x2sT")
            nc.scalar.dma_start(x2sT[:], d_x2sT)
            fwT = consts.tile([FC_IN, EPAD], bf16, name="fwT")
            nc.scalar.dma_start(fwT[:], d_fwT)

            for sp in range(NSUP):
                g0 = sp * EW
                # ---- MLP for the whole supertile ----
                h1p = pmlp.tile([S + V, EW], f32, tag="mlp", name=f"h1p_{sp}")
                nc.tensor.matmul(h1p[0:HID, :], wfc1[:], fwT[:, g0:g0 + EW], start=True, stop=True)
                h1 = postp.tile([HID, EW], bf16, tag="h1")
                silu(postp, h1, h1p[0:HID, :], bfc1[:, 0:1], "h1s", nc.gpsimd, f"h1s{sp}")
                h2p = pmlp.tile([S + V, EW], f32, tag="mlp", name=f"h2p_{sp}")
                nc.tensor.matmul(h2p[0:HID, :], wfc2[:], h1[:], start=True, stop=True)
                h2 = postp.tile([HID, EW], bf16, tag="h2")
                silu(postp, h2, h2p[0:HID, :], bfc2[:, 0:1], "h2s", nc.gpsimd, f"h2s{sp}")
                wp = pmlp.tile([S + V, EW], f32, tag="mlp", name=f"wp_{sp}")
                nc.tensor.matmul(wp[:], wfc3[:], h2[:], start=True, stop=True)
                wgt_sc = postp.tile([S, EW], bf16, tag="wgt_sc")
                nc.scalar.activation(wgt_sc[:], wp[0:S, :], Ident, bias=bfc3[0:S, 0:1])
                wgt_v = postp.tile([V, EW], bf16, tag="wgt_v")
                nc.scalar.activation(wgt_v[:], wp[S:S + V, :], Ident, bias=bfc3[S:S + V, 0:1])

                # ---- per-subtile: inputs, kron builds, B matmuls, products ----
                krons = []   # per sub: dict tag -> tile
                vecs = []    # per sub: [128, 96] bf16 (ungated vec, edge-major)
                for s in range(NSUB):
                    e0 = g0 + s * ET
                    xcat = io.tile([ET, 576], bf16, tag=f"xcat_{s}", name=f"xcat_{sp}_{s}")
                    nc.sync.dma_start(xcat[:], d_xcat[e0:e0 + ET, :])
                    x1s2 = xcat[:, 0:2 * S]
                    x2s = xcat[:, 2 * S:2 * S + S]
                    x1vg2 = xcat[:, 192:192 + 6 * V]
                    xvg = xcat[:, 384:576].rearrange("e (i q) -> e i q", q=2 * V)

                    kd = {}
                    # kron_ss [e,(u,vh,p)]
                    kd["kss"] = kronp.tile([ET, S * S], bf16, tag=f"kss_{s}", name=f"kss_{sp}_{s}")
                    nc.vector.tensor_tensor(
                        kd["kss"][:].rearrange("e (u vh p) -> e u vh p", vh=S // 2, p=2),
                        x1s2.rearrange("e (u p) -> e u p", p=2)
                            .unsqueeze(2).broadcast_to([ET, S, S // 2, 2]),
                        x2s.rearrange("e (vh p) -> e vh p", p=2)
                            .unsqueeze(1).broadcast_to([ET, S, S // 2, 2]),
                        mul_op)
                    # dot_vv = sum_i x1v_i (x) x2v_i
                    pv = [kronp.tile([ET, V * V], bf16, tag=f"pv{i}_{s}", name=f"pv{i}_{sp}_{s}") for i in range(3)]
                    for i in range(3):
                        nc.vector.tensor_tensor(
                            pv[i][:].rearrange("e (u vh p) -> e u vh p", vh=V // 2, p=2),
                            x1vg2[:, i * 2 * V:(i + 1) * 2 * V]
                                .rearrange("e (u p) -> e u p", p=2)
                                .unsqueeze(2).broadcast_to([ET, V, V // 2, 2]),
                            xvg[:, i, 0:V]
                                .rearrange("e (vh p) -> e vh p", p=2)
                                .unsqueeze(1).broadcast_to([ET, V, V // 2, 2]),
                            mul_op)
                    kd["kvv"] = kronp.tile([ET, V * V], bf16, tag=f"kvv_{s}", name=f"kvv_{sp}_{s}")
                    nc.gpsimd.tensor_tensor(kd["kvv"][:], pv[0][:], pv[1][:], add_op)
                    nc.vector.tensor_tensor(kd["kvv"][:], kd["kvv"][:], pv[2][:], add_op)
                    krons.append(kd)

                    # ---- two-stage sv/vs ----
                    HB = V * V // 2  # 512 fp32 = one PSUM bank per matmul write
                    # Bcat[e, w, 0:32]=B_sv (v), [e, w, 32:64]=B_vs (u)
                    bcat = stagep.tile([ET, V * 2 * V], bf16, tag="bcat_sb", name=f"bcat_{sp}_{s}")
                    bcat_v = bcat[:].rearrange("e (w q) -> e w q", q=2 * V)
                    bsv_p = pb.tile([ET, V * V], f32, tag="bsvh", name=f"bsv_{sp}_{s}")
                    bvs_p = pb.tile([ET, V * V], f32, tag="bvsh", name=f"bvs_{sp}_{s}")
                    for h in range(2):
                        nc.tensor.matmul(bsv_p[:, h * HB:(h + 1) * HB], x1sT[:, e0:e0 + ET],
                                         wsv[:, h * HB:(h + 1) * HB], start=True, stop=True)
                        nc.tensor.matmul(bvs_p[:, h * HB:(h + 1) * HB], x2sT[:, e0:e0 + ET],
                                         wvs[:, h * HB:(h + 1) * HB], start=True, stop=True)
                    nc.scalar.activation(
                        bcat_v[:, :, 0:V], bsv_p[:].rearrange("e (w v) -> e w v", v=V), Copy)
                    nc.scalar.activation(
                        bcat_v[:, :, V:2 * V], bvs_p[:].rearrange("e (w u) -> e w u", u=V), Copy)

                    # PP[e, i, w, q] = Bcat[e, w, q] * xvg[e, i, q]
                    ppt = ppp.tile([ET, 3 * V * 2 * V], bf16, tag=f"pp_{s}", name=f"pp_{sp}_{s}")
                    ppv = ppt[:].rearrange("e (i w q) -> e i w q", i=3, w=V, q=2 * V)
                    for i in range(3):
                        eng = nc.vector if i != 1 else nc.gpsimd
                        eng.tensor_tensor(
                            ppv[:, i, :, :],
                            bcat_v,
                            xvg[:, i, :].unsqueeze(1).broadcast_to([ET, V, 2 * V]),
                            mul_op)
                    # adder tree over q (tensor_tensor stays in DVE 2x mode;
                    # tensor_reduce runs at 1x)
                    tree = ppp.tile([ET, 3 * V * 62], bf16, tag=f"tree_{s}", name=f"tree_{sp}_{s}")
                    w_cur = 2 * V
                    cur = ppt[:].rearrange("e (iw q) -> e iw q", q=2 * V)
                    toff = 0
                    while w_cur > 2:
                        h = w_cur // 2
                        nxt = tree[:, toff:toff + 3 * V * h].rearrange(
                            "e (iw q) -> e iw q", q=h)
                        nc.vector.tensor_tensor(nxt, cur[:, :, 0:h], cur[:, :, h:w_cur], add_op)
                        cur = nxt
                        toff += 3 * V * h
                        w_cur = h
                    vec = io.tile([ET, 3 * V], bf16, tag=f"vec_{s}", name=f"vec_{sp}_{s}")
                    nc.vector.tensor_tensor(vec[:], cur[:, :, 0], cur[:, :, 1], add_op)
                    vecs.append(vec)

                # ---- ss/vv chunk loop: transpose -> copy -> matmul ----
                acc_ss = pacc.tile([S + V, EW], f32, tag="acc_ss", name=f"acc_ss_{sp}")
                for g in range(0, NCH, CPG):
                    sb = stagep.tile([128, CPG * EW], bf16, tag="stage_sb", name=f"sb_{sp}_{g}")
                    st = pst.tile([128, CPG * EW], bf16, tag="stage", name=f"st_{sp}_{g}")
                    for j in range(CPG):
                        tag, koff = chunk_meta[g + j][0], chunk_meta[g + j][1]
                        for s in range(NSUB):
                            nc.tensor.transpose(
                                st[:, (j * NSUB + s) * ET:(j * NSUB + s + 1) * ET],
                                krons[s][tag][:, koff:koff + 128], ident[:])
                    nc.scalar.activation(sb[:], st[:], Copy)
                    for j in range(CPG):
                        c = g + j
                        woff = chunk_meta[c][2]
                        nc.tensor.matmul(
                            acc_ss[:], wssvv[:, woff:woff + S + V],
                            sb[:, j * EW:(j + 1) * EW],
                            start=(c == 0), stop=(c == NCH - 1))

                # ---- gate + elementwise conv ----
                silu_sc = postp.tile([S, EW], bf16, tag="silu_sc")
                silu(postp, silu_sc, acc_ss[0:S, :], None, "scs", nc.vector, f"scs{sp}")
                sig = postp.tile([V, EW], bf16, tag="sig")
                nc.scalar.activation(sig[:], acc_ss[S:S + V, :], Sigm)
                sgw = postp.tile([V, EW], bf16, tag="sgw")
                nc.vector.tensor_tensor(sgw[:], sig[:], wgt_v[:], mul_op)

                osc = postp.tile([S, EW], f32, tag="osc")
                nc.gpsimd.tensor_tensor(osc[:], silu_sc[:], wgt_sc[:], mul_op)
                nc.sync.dma_start(d_osc[:, g0:g0 + EW], osc[:])

                for s in range(NSUB):
                    e0 = g0 + s * ET
                    sgwT_p = pst.tile([128, CPG * EW], bf16, tag="stage", name=f"sgwT_{sp}_{s}")
                    nc.tensor.transpose(sgwT_p[:, 0:V], sgw[:, s * ET:(s + 1) * ET], ident[0:V, 0:V])
                    ovec = postp.tile([ET, 3 * V], f32, tag=f"ovec_{s}", name=f"ovec_{sp}_{s}")
                    nc.vector.tensor_tensor(
                        ovec[:].rearrange("e (i w) -> e i w", i=3),
                        vecs[s][:].rearrange("e (i w) -> e i w", i=3),
                        sgwT_p[:, 0:V].unsqueeze(1).broadcast_to([ET, 3, V]),
                        mul_op)
                    nc.sync.dma_start(d_ovec[e0:e0 + ET, :], ovec[:])

    nc.compile()
    return nc


_CACHED = {}


def kernel(fea_in1, fea_in2, fea_weight,
           w_ss_s, w_vv_s, w_ss_g, w_vv_g, w_sv_v, w_vs_v,
           fc_w1, fc_b1, fc_w2, fc_b2, fc_w3, fc_b3, batch_edge):
    fea_in1 = np.asarray(fea_in1, dtype=np.float32)
    fea_in2 = np.asarray(fea_in2, dtype=np.float32)
    fea_weight = np.asarray(fea_weight, dtype=np.float32)

    wd = _prep_weights(np.asarray(w_ss_s, np.float32), np.asarray(w_vv_s, np.float32),
                       np.asarray(w_ss_g, np.float32), np.asarray(w_vv_g, np.float32),
                       np.asarray(w_sv_v, np.float32), np.asarray(w_vs_v, np.float32),
                       np.asarray(fc_w1, np.float32), np.asarray(fc_b1, np.float32),
                       np.asarray(fc_w2, np.float32), np.asarray(fc_b2, np.float32),
                       np.asarray(fc_w3, np.float32), np.asarray(fc_b3, np.float32))

    if "nc" not in _CACHED:
        _CACHED["nc"] = _build_program()
    nc = _CACHED["nc"]

    in_maps = []
    for c in range(NCORES):
        s0 = c * EC
        f1 = np.zeros((EPAD, 160), np.float32)
        f1[:EC] = fea_in1[s0:s0 + EC]
        f2 = np.zeros((EPAD, 160), np.float32)
        f2[:EC] = fea_in2[s0:s0 + EC]
        x1s = f1[:, :S]
        x2s = f2[:, :S]
        x1v = f1[:, S:].reshape(EPAD, V, 3)   # [e, u, i]
        x2v = f2[:, S:].reshape(EPAD, V, 3)
        x1vg = np.ascontiguousarray(x1v.transpose(0, 2, 1))  # [e, i, u]
        x2vg = np.ascontiguousarray(x2v.transpose(0, 2, 1))  # [e, i, v]
        fwT = np.zeros((FC_IN, EPAD), BF16)
        fwT[:, :EC] = fea_weight[s0:s0 + EC].T.astype(BF16)
        xvgcat = np.concatenate([x2vg, x1vg], axis=2)  # [e, 3, 64]
        xcat = np.concatenate([
            np.repeat(x1s, 2, axis=1),                 # [e, 128]
            x2s,                                       # [e, 64]
            np.repeat(x1vg.reshape(EPAD, 3 * V), 2, axis=1),  # [e, 192]
            xvgcat.reshape(EPAD, 6 * V),               # [e, 192]
        ], axis=1).astype(BF16)
        m = {
            "xcat": xcat,
            "x1sT": np.ascontiguousarray(x1s.T).astype(BF16),
            "x2sT": np.ascontiguousarray(x2s.T).astype(BF16),
            "fwT": fwT,
        }
        m.update(wd)
        in_maps.append(m)

    import os
    trace = bool(int(os.environ.get("KERNEL_TRACE", "0")))
    res = run_bass_kernel_spmd(nc, in_maps, core_ids=list(range(NCORES)), trace=trace)
    _CACHED["exec_time_ns"] = res.exec_time_ns

    out = np.empty((E, S + 3 * V), np.float32)
    # ovec col (i*32+w) -> output column 64 + 3*w + i
    vec_cols = np.empty(3 * V, np.int64)
    for i in range(3):
        for w in range(V):
            vec_cols[i * V + w] = S + 3 * w + i
    for c in range(NCORES):
        s0 = c * EC
        osc = np.asarray(res.results[c]["out_sc"])[:, :EC]     # [64, EC]
        ovec = np.asarray(res.results[c]["out_vec"])[:EC, :]   # [EC, 96]
        out[s0:s0 + EC, :S] = osc.T
        out[s0:s0 + EC, vec_cols] = ovec
    return out


if __name__ == "__main__":
    rng = np.random.default_rng(0)
    ins = {
        "fea_in1": rng.standard_normal((E, 160)).astype(np.float32),
        "fea_in2": rng.standard_normal((E, 160)).astype(np.float32),
        "fea_weight": rng.standard_normal((E, FC_IN)).astype(np.float32),
        "w_ss_s": rng.standard_normal((S, S, S)).astype(np.float32),
        "w_vv_s": rng.standard_normal((V, V, S)).astype(np.float32),
        "w_ss_g": rng.standard_normal((S, S, V)).astype(np.float32),
        "w_vv_g": rng.standard_normal((V, V, V)).astype(np.float32),
        "w_sv_v": rng.standard_normal((S, V, V)).astype(np.float32),
        "w_vs_v": rng.standard_normal((V, S, V)).astype(np.float32),
        "fc_w1": rng.standard_normal((FC_IN, HID)).astype(np.float32),
        "fc_b1": np.zeros(HID, np.float32),
        "fc_w2": rng.standard_normal((HID, HID)).astype(np.float32),
        "fc_b2": np.zeros(HID, np.float32),
        "fc_w3": rng.standard_normal((HID, S + V)).astype(np.float32),
        "fc_b3": np.zeros(S + V, np.float32),
        "batch_edge": np.zeros(E, np.int32),
    }
    out = kernel(**ins)
    print("kernel out", out.shape, out.dtype, float(np.abs(out).mean()))
